# revision 73
# baseline (speedup 1.0000x reference)
"""Trainium2 Bass kernel for nn_Attn_48052094107916 (sparse_attention).

Math (per batch b):
  q = x @ Wq.T -> [N, 4, 16];  k = x @ Wk.T -> [N, 4, 16];  v = x @ Wv.T -> [N, 8, 16]
  attn[g,i,j] = <q[i,g,:], k[j,g,:]>
  mw[i,j,g,l] = (masks @ mask_proj)[i,j,g*8+l]
  scores[l,i,j] = sum_g attn[g,i,j] * mw[i,j,g,l]
  out[i,l,:]  = softmax_j(scores[l,i,:]) @ v[:,l,:]

Key restructuring: using mask_proj's rank-3 structure,
  scores[l] = sum_m masks_m (x) w_{m,l},   w_{m,l} = sum_g P[m,g,l] attn_g
and w is computed DIRECTLY on the TensorEngine by scaling q into 24 virtual
heads (contraction 64): w[m,l][j,i] = <k[j,:], qtilde[m,l][i,:]>.

Engine plan (engine menu on TRN2: GPSIMD/Pool cannot touch PSUM, ACT cannot
multiply tensors, TensorScalarPtr has no DVE fast modes, TensorTensor gets
2x only for all-SBUF packed 16-bit, psum reads are always 1x):
  - per (b, key-chunk) iteration, w is built per l-half: m0,m1 into a
    double-buffered 2-bank psum tile, m2 (both halves) into a single
    2-bank tile.  m0,m1 convert psum->sbuf fp16 in ONE ACT copy per half;
    the masks product runs as 2x TensorTensor on Pool (h0) / DVE (h1);
    m2 is multiplied straight from PSUM on DVE at 1x, skipping its copy.
  - adds s12/sc on DVE (short critical chain into exp).
  - exp on ACT, batched 4 iterations per instruction (2 for the last 4,
    shortening the tail) to amortize ACT's fixed access cost.
  - PV is TRANSPOSED: stationary = probs[:, l, :], moving = v17 (16 v cols
    + ones column) so the accumulator lands as pv[i, l, d|den] in ONE psum
    bank; the softmax division is a reciprocal + multiply and the output
    DMA is a plain [128, 128] store (no 32x32 transpose epilogue).
  - software-pipelined emission: PV matmuls of older iterations are
    drained in groups of 4 between w-matmul groups (never head-blocking
    the PE's 4-deep wait queue); projections of batch b+1 are emitted
    inside batch b's main loop; batch 0 splits its kT copy / q-scaling
    for a fast start; pcol_rep is only i-period-2 (packed last dim keeps
    the 2x mode, resident table is 6KB instead of 400KB).

Sharding: 8 cores, core r owns query rows [128r, 128r+128) for ALL batches
(sequence parallel).  No collectives.  Cost-model exec: ~225.6us/core
(ACT 211us busy ~94%, DVE 207us, Pool 137us, PE 95us) vs 287.8us baseline.
"""

import os
import sys

import numpy as np

sys.path.insert(0, "/opt/trn_rl_repo")

B, N, C = 8, 1024, 128
G, L, HD = 4, 8, 16
NCORES = 8
RQ = N // NCORES  # query rows per core = 128
NCH = N // 128  # key chunks = 8

_cache = {}


def _build():
    import os as _os
    import concourse.bacc as bacc
    import concourse.tile as tile
    from concourse import mybir

    f32 = mybir.dt.float32
    bf16 = mybir.dt.bfloat16
    fp16 = mybir.dt.float16
    AF = mybir.ActivationFunctionType
    OP = mybir.AluOpType

    nc = bacc.Bacc("TRN2", target_bir_lowering=False)

    xt_d = nc.dram_tensor("xt", [B, C, N], fp16, kind="ExternalInput")
    xqt_d = nc.dram_tensor("xqt", [B, C, RQ], fp16, kind="ExternalInput")
    mt_d = nc.dram_tensor("maskst", [NCH, 128, 3, 128], fp16, kind="ExternalInput")
    wqt_d = nc.dram_tensor("wqt", [C, 64], fp16, kind="ExternalInput")
    wkt_d = nc.dram_tensor("wkt", [C, 64], fp16, kind="ExternalInput")
    wvt_d = nc.dram_tensor("wvt", [C, C], fp16, kind="ExternalInput")
    pcolr_d = nc.dram_tensor("pcolr", [64, 3, L, 2], fp16, kind="ExternalInput")
    out_d = nc.dram_tensor("out", [B, RQ, C], f32, kind="ExternalOutput")

    with tile.TileContext(nc) as tc, tc.tile_pool(name="singles", bufs=1) as singles, \
            tc.tile_pool(name="xtb", bufs=2) as xtb_pool, \
            tc.tile_pool(name="small", bufs=3) as small, \
            tc.tile_pool(name="wsb", bufs=int(os.environ.get("KBUF", "5"))) as wsb_pool, \
            tc.tile_pool(name="prod", bufs=int(os.environ.get("KBUF", "5"))) as prod, \
            tc.tile_pool(name="probs", bufs=int(os.environ.get("KPBUF", "2"))) as probs_pool, \
            tc.tile_pool(name="epi", bufs=2) as epi, \
            tc.tile_pool(name="w_ps", bufs=2, space="PSUM") as w_ps_pool, \
            tc.tile_pool(name="m2_ps", bufs=1, space="PSUM") as m2_ps_pool, \
            tc.tile_pool(name="pv_ps", bufs=2, space="PSUM") as pv_ps:

        # ---------------- resident tensors ----------------
        # batch 0's x DMAs go FIRST: the very first w-matmuls wait on them,
        # while the (small) weight loads overlap with the kT matmul setup
        xtb0 = {}

        def xt_dma(b):
            xT = xtb_pool.tile([C, N], fp16, tag="xT", name="xT")
            for h in range(2):
                nc.sync.dma_start(out=xT[:, h * 512:(h + 1) * 512],
                                  in_=xt_d[b, :, h * 512:(h + 1) * 512])
            return xT

        wqt = singles.tile([C, 64], fp16)
        wkt = singles.tile([C, 64], fp16)
        wvt = singles.tile([C, C], fp16)
        xqT = singles.tile([C, B, RQ], fp16)
        nc.sync.dma_start(out=xqT[:, 0], in_=xqt_d[0])
        xtb0[0] = xt_dma(0)
        nc.sync.dma_start(out=wkt, in_=wkt_d[:, :])
        nc.sync.dma_start(out=wqt, in_=wqt_d[:, :])
        nc.sync.dma_start(out=wvt, in_=wvt_d[:, :])

        pcolr = singles.tile([64, 3, L, 2], fp16)
        nc.sync.dma_start(out=pcolr, in_=pcolr_d[:, :, :, :])

        masksT = singles.tile([128, NCH, 3, 128], fp16)  # [j, ch, m, i]
        kT = singles.tile([64, B, N], fp16)
        qtb = singles.tile([64, B, 3, L, RQ], fp16)  # P-scaled q, 24 virtual heads
        v17 = singles.tile([128, B, NCH, L, 17], bf16)  # [j, b, ch, l, d|ones]
        nc.gpsimd.memset(v17[:, :, :, :, 16:17], 1.0)

        # half-iteration psum tiles, double-buffered: m0,m1 [128, 1024]
        # (2 banks, released by the ACT copy alone) and m2 [128, 512]
        # (1 bank, released by the DVE psum-mult alone)
        def wp_tile():
            return w_ps_pool.tile([128, 2 * 4 * RQ], f32, tag="wp", name="wp")

        def m2p_tile():
            return m2_ps_pool.tile([128, 8 * RQ], f32, tag="m2p", name="m2p")

        # ---------------- per-batch projections ----------------
        def proj(b):
            if b in xtb0:
                xT = xtb0[b]
            else:
                xT = xt_dma(b)
                nc.sync.dma_start(out=xqT[:, b], in_=xqt_d[b])

            wpA = wp_tile()
            # kT[b] = wkt.T @ xT   [64, N]; both halves, ONE copy (two for
            # batch 0, so the first main iteration unblocks sooner)
            for h in range(2):
                nc.tensor.matmul(wpA[0:64, h * 512:(h + 1) * 512], wkt,
                                 xT[:, h * 512:(h + 1) * 512],
                                 start=True, stop=True)
                if b == 0:
                    nc.scalar.copy(out=kT[:, b, h * 512:(h + 1) * 512],
                                   in_=wpA[0:64, h * 512:(h + 1) * 512])
            if b > 0:
                nc.scalar.copy(out=kT[:, b], in_=wpA[0:64, 0:1024])

            # qT[b] = wqt.T @ xqT[b] [64, RQ]; one 2x TT scales it into the
            # 24 virtual heads against host-replicated pcol_rep
            qps = m2p_tile()[0:64, 0:RQ]
            nc.tensor.matmul(qps, wqt, xqT[:, b, :], start=True, stop=True)
            qt_sb = small.tile([64, RQ], fp16, tag="qt", name="qt_sb")
            if _os.environ.get("KQTC", "0") == "1":
                nc.vector.tensor_copy(out=qt_sb, in_=qps)
            else:
                nc.scalar.copy(out=qt_sb, in_=qps)
            # in1 repeats pcolr's 2-wide innermost across i (stride-0
            # middle dim); all operands keep a packed >=2 last dim, so the
            # TT still runs at 2x with only a [64,3,L,2] resident table
            if b == 0:
                # finer grain for batch 0 so the first w-matmuls unblock
                for m in range(3):
                    for lh in range(2):
                        nc.vector.tensor_tensor(
                            out=qtb[:, 0, m, 4 * lh:4 * lh + 4]
                            .rearrange("p l (r t) -> p l r t", t=2),
                            in0=qt_sb[:, None, :]
                            .to_broadcast((64, 4, RQ))
                            .rearrange("p l (r t) -> p l r t", t=2),
                            in1=pcolr[:, m, 4 * lh:4 * lh + 4, None, :]
                            .to_broadcast((64, 4, RQ // 2, 2)),
                            op=OP.mult,
                        )
            else:
                nc.vector.tensor_tensor(
                    out=qtb[:, b]
                    .rearrange("p m l (r t) -> p (m l) r t", t=2),
                    in0=qt_sb[:, None, :]
                    .to_broadcast((64, 3 * L, RQ))
                    .rearrange("p q (r t) -> p q r t", t=2),
                    in1=pcolr[:, :, :, None, :]
                    .to_broadcast((64, 3, L, RQ // 2, 2))
                    .rearrange("p m l r t -> p (m l) r t"),
                    op=OP.mult,
                )

            # v[b]: v = x @ Wv.T -> v17, copies batched 4 key-chunks at a time
            wpB = wp_tile()
            for ch in range(NCH):
                base = 512 if ch >= 4 else 0
                ps = wpB[:, base + (ch % 4) * 128:base + (ch % 4 + 1) * 128]
                nc.tensor.matmul(ps, xT[:, ch * 128:(ch + 1) * 128], wvt,
                                 start=True, stop=True)
                if ch == NCH - 1:
                    nc.scalar.copy(
                        out=v17[:, b, :, :, 0:16],
                        in_=wpB[:, 0:1024]
                        .rearrange("p (c l d) -> p c l d", c=8, l=L),
                    )

        lazy = _os.environ.get("KLAZY", "1") == "1"
        proj(0)
        if not lazy:
            for b in range(1, B):
                proj(b)
        for ch in range(NCH):
            nc.sync.dma_start(out=masksT[:, ch], in_=mt_d[ch])

        # ---------------- main loop (software-pipelined emission) --------
        pv_tiles = {}
        pvq = []  # pending ('mm', closure) / ('epi', b) items

        def emit_pv(b, ch, pb):
            # transposed PV: stationary = pb[:, l, :] (128x128), moving =
            # v17 (17 cols incl the ones/denominator column).  Output lands
            # as pv'[i, l, d|den] in a SINGLE psum bank, so the epilogue
            # needs no 32x32 transpose and pv double-buffers.
            def one(l):
                def f():
                    if b not in pv_tiles:
                        pv_tiles[b] = pv_ps.tile([128, L, 17], f32, tag="pv",
                                                 name="pv")
                    nc.tensor.matmul(
                        pv_tiles[b][:, l, :],
                        pb[:, l, :],
                        v17[:, b, ch, l, :],
                        start=(ch == 0 and l == 0), stop=(ch == NCH - 1),
                        skip_group_check=True,
                    )
                return f
            for l in range(L):
                pvq.append(("mm", one(l)))
            if ch == NCH - 1:
                pvq.append(("epi", b))

        def drain_pv(n):
            done = 0
            while pvq and done < n:
                kind, payload = pvq[0]
                if kind == "mm":
                    payload()
                    done += 1
                else:
                    epilogue(payload)
                pvq.pop(0)
            # a batch's epilogue goes out with its last PV matmul so the
            # transpose (which releases the single pv psum buffer) lands
            # early in the DVE queue
            if pvq and pvq[0][0] == "epi":
                epilogue(pvq.pop(0)[1])

        def epilogue(b):
            # normalize: out[i, l, d] = pv'[i, l, d] / pv'[i, l, 16]
            pv = pv_tiles.pop(b)
            denr = epi.tile([128, L], f32, tag="denr")
            nc.vector.reciprocal(out=denr, in_=pv[:, :, 16])
            ob = epi.tile([128, L, 16], f32, tag="ob")
            nc.vector.tensor_tensor(
                out=ob,
                in0=pv[:, :, 0:16],
                in1=denr[:, :, None].to_broadcast((128, L, 16)),
                op=OP.mult,
            )
            nc.sync.dma_start(out=out_d[b],
                              in_=ob.rearrange("p l d -> p (l d)"))

        # Software pipeline, 3 stages deep: iteration k emits its own
        # matmuls+copies+mults, iteration k-1's adds+exp, and iteration
        # k-2's PV -- so every queued instruction is ready (or nearly so)
        # when it reaches the head of its engine's 4-deep wait queue.
        def stage_mults(b, ch):
            prm = prod.tile([128, 3, L, RQ], fp16, tag="prm")
            m2p = m2p_tile()
            for hf in range(2):
                wp = wp_tile()
                lsl = slice(4 * hf, 4 * hf + 4)
                for m in range(2):
                    nc.tensor.matmul(
                        wp[:, m * 512:(m + 1) * 512],
                        kT[:, b, ch * 128:(ch + 1) * 128],
                        qtb[:, b, m, lsl].rearrange("p l i -> p (l i)"),
                        start=True, stop=True,
                    )
                nc.tensor.matmul(
                    m2p[:, hf * 512:hf * 512 + 512],
                    kT[:, b, ch * 128:(ch + 1) * 128],
                    qtb[:, b, 2, lsl].rearrange("p l i -> p (l i)"),
                    start=True, stop=True,
                )
                drain_pv(int(_os.environ.get('KDRAIN', '4')))
                # m0,m1: psum->sbuf fp16 in one ACT copy
                w_sb = wsb_pool.tile([128, 2, 4, RQ], fp16, tag="wsb")
                nc.scalar.copy(
                    out=w_sb.rearrange("p m l i -> p (m l i)"),
                    in_=wp[:, 0:1024])
                # m0,m1 from sbuf fp16: h0 on Pool (its consumer, the adds,
                # comes late enough to hide Pool latency), h1 on DVE
                eng = nc.gpsimd if hf == 0 else nc.vector
                eng.tensor_tensor(
                    out=prm[:, 0:2, lsl],
                    in0=masksT[:, ch, 0:2, None, :]
                    .to_broadcast((128, 2, 4, RQ)),
                    in1=w_sb,
                    op=OP.mult,
                )
            # m2 (both halves) multiplied straight from PSUM in ONE 1x op
            nc.vector.tensor_tensor(
                out=prm[:, 2],
                in0=masksT[:, ch, 2, None, :].to_broadcast((128, L, RQ)),
                in1=m2p.rearrange("p (l i) -> p l i", l=L),
                op=OP.mult,
            )
            return prm

        # iterations are processed in groups of EXN sharing one sc/probs
        # tile and ONE exp instruction (cuts ACT's per-instruction tax)
        EXN = int(_os.environ.get("KEXPN", "4"))
        pair = {}

        def stage_adds(k, prm):
            # last 4 iterations use pair-sized exp groups: shorter tail
            exn = 2 if k >= 60 else EXN
            j = k % EXN if exn == EXN else (k % 2)
            if j == 0:
                pair["sc"] = prod.tile([128, exn, L, RQ], fp16, tag="sc",
                                       name="sc2")
                pair["pb"] = probs_pool.tile([128, exn, L, RQ], bf16,
                                             tag="probs", name="pb2")
            sc2, pb2 = pair["sc"], pair["pb"]
            s12 = prod.tile([128, L, RQ], fp16, tag="s12")
            nc.vector.tensor_tensor(out=s12, in0=prm[:, 1], in1=prm[:, 2],
                                    op=OP.add)
            nc.vector.tensor_tensor(out=sc2[:, j], in0=prm[:, 0],
                                    in1=s12, op=OP.add)
            if j == exn - 1:
                nc.scalar.activation(out=pb2, in_=sc2, func=AF.Exp)
            return pb2, exn, j

        iters = [(b, ch) for b in range(B) for ch in range(NCH)]
        for k, (b, ch) in enumerate(iters):
            if lazy and ch == int(_os.environ.get('KLCH', '2')) and b + 1 < B:
                proj(b + 1)
            prm = stage_mults(b, ch)
            pb2, exn, j = stage_adds(k, prm)
            if j == exn - 1:
                for jj in range(exn):
                    bp, chp = iters[k - exn + 1 + jj]
                    emit_pv(bp, chp, pb2[:, jj])
        while pvq:
            drain_pv(8)

    nc.compile()
    return nc


def _get_graph():
    if "nc" not in _cache:
        _cache["nc"] = _build()
    return _cache["nc"]


def kernel(x, masks, Wq, Wk, Wv, mask_proj):
    from concourse import bass_utils

    x = np.asarray(x, dtype=np.float32)
    masks = np.asarray(masks, dtype=np.float32)
    Wq = np.asarray(Wq, dtype=np.float32)
    Wk = np.asarray(Wk, dtype=np.float32)
    Wv = np.asarray(Wv, dtype=np.float32)
    mask_proj = np.asarray(mask_proj, dtype=np.float32)

    f16 = np.float16
    xt = np.ascontiguousarray(x.transpose(0, 2, 1)).astype(f16)  # [B, C, N]
    wqt = np.ascontiguousarray(Wq.T).astype(f16)
    wkt = np.ascontiguousarray(Wk.T).astype(f16)
    wvt = np.ascontiguousarray(Wv.T).astype(f16)
    # pcolr[gd, m, l, i] = mask_proj[m, g(gd)*L + l]  (replicated over i)
    g_of = np.arange(64) // HD
    pcol = np.empty((64, 3, L), dtype=np.float32)
    for gd in range(64):
        for m in range(3):
            for l in range(L):
                pcol[gd, m, l] = mask_proj[m, g_of[gd] * L + l]
    pcolr = np.ascontiguousarray(
        np.broadcast_to(pcol[:, :, :, None], (64, 3, L, 2))).astype(f16)

    in_maps = []
    for r in range(NCORES):
        sl = slice(r * RQ, (r + 1) * RQ)
        # maskst[ch, j, m, i] = masks[r*128+i, ch*128+j, m]
        msl = masks[sl]  # [i=128, N, 3]
        mt = np.ascontiguousarray(
            msl.reshape(RQ, NCH, 128, 3).transpose(1, 2, 3, 0)).astype(f16)
        in_maps.append({
            "xt": xt,
            "xqt": np.ascontiguousarray(xt[:, :, sl]),
            "maskst": mt,
            "wqt": wqt, "wkt": wkt, "wvt": wvt, "pcolr": pcolr,
        })

    nc = _get_graph()
    trace = bool(int(os.environ.get("KBENCH_TRACE", "0")))
    try:
        res = bass_utils.run_bass_kernel_spmd(
            nc, in_maps, core_ids=list(range(NCORES)), trace=trace,
        )
    except (ImportError, ModuleNotFoundError):
        # NTFF profile hook unavailable in this environment; run untraced
        res = bass_utils.run_bass_kernel_spmd(
            nc, in_maps, core_ids=list(range(NCORES)), trace=False,
        )
    _cache["last_exec_time_ns"] = getattr(res, "exec_time_ns", None)

    out = np.empty((B, N, C), dtype=np.float32)
    for r in range(NCORES):
        out[:, r * RQ:(r + 1) * RQ, :] = res.results[r]["out"]
    return out
